# revision 1
# baseline (speedup 1.0000x reference)
"""Trainium2 Bass kernel for BaseLinearLayerWithLoRA (moe_routing).

out = x @ W^T + b  +  per-token LoRA:  out[t] += (x[t] @ A[l]^T) @ B[l]^T,  l = idx[t]

Sharding: data-parallel over tokens across 8 NeuronCores (4096 tokens each);
W, bias and the stacked LoRA A/B tables are replicated.

Per-core kernel design (single pass over tokens):
  - Base GEMM runs in float32r (full-rate PE streaming at moving-dim 512,
    ~1.5e-4 rms error vs fp32).  lhsT = x^T chunks (host-transposed x),
    rhs = W^T (host-transposed W) fully resident in SBUF (128KB/partition);
    x is streamed once in 256-token super-blocks, with a 4-wide o-sweep
    per stationary chunk into 4 PSUM banks.
  - LoRA shrink: S^T[r,t] = sum_d A_all^T[d,r] x^T[d,t] for all 8 adapters
    at once (A_all = stacked [128, 2048]), one fp32r GEMM per super-block.
    A host-precomputed one-hot mask (mask[r,t] = r//16==idx[t]) zeroes the
    rows of foreign adapters per token column (DVE multiply, cast to bf16).
    The expand is then a single bf16 matmul per output tile contracting all
    128 adapter-rank rows, accumulated into the same PSUM tile as the base
    GEMM.
  - Bias is added during the PSUM->SBUF drain (host-replicated to 128 rows,
    cast to bf16 on load).
"""

import sys

for _p in ("/opt/trn_rl_repo", "/root/.axon_site/_ro/trn_rl_repo"):
    if _p not in sys.path:
        sys.path.insert(0, _p)

import numpy as np
import ml_dtypes

import concourse.bass as bass  # noqa: F401  (registers engines)
import concourse.mybir as mybir
import concourse.tile as tile
from concourse import bacc
from concourse.bass_utils import run_bass_kernel_spmd

N_CORES = 8
T_FULL, D_IN, D_OUT = 32768, 2048, 2048
MAX_LORAS, RANK = 8, 16
T_CORE = T_FULL // N_CORES          # 4096 tokens per core
SB_T = 256                          # super-block tokens
N_SB = T_CORE // SB_T               # 16 super-blocks
N_BLK = SB_T // 128                 # 2 token blocks per super-block
KC = D_IN // 128                    # 16 contraction chunks
N_OT = D_OUT // 512                 # 4 o-tiles (full width resident)

_CACHED = {}


def _build():
    if "nc" in _CACHED:
        return _CACHED["nc"]

    f32 = mybir.dt.float32
    f32r = mybir.dt.float32r
    bf16 = mybir.dt.bfloat16

    nc = bacc.Bacc("TRN2", target_bir_lowering=False, debug=False)

    xT = nc.dram_tensor("xT", [D_IN, T_CORE], f32r, kind="ExternalInput")
    wT = nc.dram_tensor("wT", [D_IN, D_OUT], f32r, kind="ExternalInput")
    aT = nc.dram_tensor("aT", [D_IN, 128], f32r, kind="ExternalInput")
    bT = nc.dram_tensor("bT", [128, D_OUT], bf16, kind="ExternalInput")
    maskM = nc.dram_tensor("maskM", [128, T_CORE], bf16, kind="ExternalInput")
    bias_rep = nc.dram_tensor("bias_rep", [128, D_OUT], f32, kind="ExternalInput")
    out = nc.dram_tensor("out", [T_CORE, D_OUT], f32, kind="ExternalOutput")

    xT_v = xT.rearrange("(c p) t -> p c t", p=128)      # [128, 16, T_CORE]
    wT_v = wT.rearrange("(c p) o -> p c o", p=128)      # [128, 16, 2048]
    aT_v = aT.rearrange("(c p) r -> p c r", p=128)      # [128, 16, 128]

    with tile.TileContext(nc) as tc:
        with (
            tc.tile_pool(name="const", bufs=1) as const,
            tc.tile_pool(name="wpool", bufs=1) as wpool,
            tc.tile_pool(name="xpool", bufs=2) as xpool,
            tc.tile_pool(name="mpool", bufs=2) as mpool,
            tc.tile_pool(name="opool", bufs=3) as opool,
            tc.tile_pool(name="pso", bufs=8, space="PSUM") as pso,
        ):
            at = const.tile([128, KC, 128], f32r)
            bt = const.tile([128, D_OUT], bf16)
            bias_t = const.tile([128, D_OUT], bf16)
            wt = wpool.tile([128, KC, D_OUT], f32r)
            for c in range(KC):
                # W loads ride the second HWDGE ring (ACT) so they don't
                # head-of-line-block the x/mask stream on the SP ring.
                nc.scalar.dma_start(wt[:, c, :], wT_v[:, c, :])
            for s in range(N_SB):
                t0 = s * SB_T
                xt = xpool.tile([128, KC, SB_T], f32r, tag="xt", name="xt")
                # per-block pieces so block-0 matmuls can start after ~1MB
                for b4 in range(N_BLK):
                    nc.sync.dma_start(
                        xt[:, :, b4 * 128:(b4 + 1) * 128],
                        xT_v[:, :, t0 + b4 * 128:t0 + (b4 + 1) * 128])
                mk = mpool.tile([128, SB_T], bf16, tag="mk", name="mk")
                if s == 0:
                    nc.sync.dma_start(at[:], aT_v[:])
                nc.sync.dma_start(mk[:], maskM[:, t0:t0 + SB_T])
                if s == 0:
                    nc.sync.dma_start(bt[:], bT[:])
                    nc.gpsimd.dma_start(bias_t[:], bias_rep[:])  # SWDGE cast f32->bf16
                s_m = mpool.tile([128, SB_T], bf16, tag="s_m", name="s_m")
                for b in range(N_BLK):
                    tb = b * 128
                    psums = [
                        pso.tile([128, 512], f32, tag="ps_o", name=f"ps_o{o}")
                        for o in range(N_OT)
                    ]
                    for c in range(KC):
                        for o in range(N_OT):
                            nc.tensor.matmul(
                                psums[o][:],
                                xt[:, c, tb:tb + 128],
                                wt[:, c, o * 512:(o + 1) * 512],
                                start=(c == 0), stop=False)
                    if b == 0:
                        # LoRA shrink for the whole super-block (all adapters),
                        # emitted after block-0 base matmuls so the PE has work
                        # while at/mask are still in flight.
                        ps_s = pso.tile([128, 512], f32, tag="ps_o", name="ps_s")
                        for c in range(KC):
                            nc.tensor.matmul(ps_s[:, :SB_T], at[:, c, :], xt[:, c, :],
                                             start=(c == 0), stop=(c == KC - 1))
                        nc.vector.tensor_tensor(s_m[:], ps_s[:, :SB_T], mk[:],
                                                mybir.AluOpType.mult)
                    for o in range(N_OT):
                        nc.tensor.matmul(
                            psums[o][:],
                            s_m[:, tb:tb + 128],
                            bt[:, o * 512:(o + 1) * 512],
                            start=False, stop=True)
                    half = D_OUT // 2
                    ot = opool.tile([128, half], f32, tag="ot", name="ot")
                    for o in range(2):
                        nc.vector.tensor_tensor(
                            ot[:, o * 512:(o + 1) * 512], psums[o][:],
                            bias_t[:, o * 512:(o + 1) * 512],
                            mybir.AluOpType.add)
                    nc.sync.dma_start(out[t0 + tb:t0 + tb + 128, :half], ot[:])
                    ot2 = opool.tile([128, half], f32, tag="ot", name="ot2")
                    for o in range(2, 4):
                        nc.vector.tensor_tensor(
                            ot2[:, (o - 2) * 512:(o - 1) * 512], psums[o][:],
                            bias_t[:, o * 512:(o + 1) * 512],
                            mybir.AluOpType.add)
                    nc.sync.dma_start(out[t0 + tb:t0 + tb + 128, half:], ot2[:])

    nc.compile()
    _CACHED["nc"] = nc
    return nc


def _prep_inputs(x, base_weight, base_bias, lora_a, lora_b, token_lora_indices):
    bf16 = ml_dtypes.bfloat16
    x = np.asarray(x, dtype=np.float32)
    w = np.asarray(base_weight, dtype=np.float32)
    bias = np.asarray(base_bias, dtype=np.float32)
    la = np.asarray(lora_a, dtype=np.float32)
    lb = np.asarray(lora_b, dtype=np.float32)
    idx = np.asarray(token_lora_indices, dtype=np.int32)

    wT = np.ascontiguousarray(w.T)                                   # [D_IN, D_OUT]
    aT = np.ascontiguousarray(la.reshape(128, D_IN).T)               # [D_IN, 128]
    bT = np.ascontiguousarray(
        lb[:, 0].transpose(0, 2, 1).reshape(128, D_OUT)).astype(bf16)
    bias_rep = np.ascontiguousarray(
        np.broadcast_to(bias[None, :], (128, D_OUT)))                # [128, D_OUT]
    mask = (np.arange(128, dtype=np.int32)[:, None] // RANK
            == idx[None, :]).astype(bf16)                            # [128, T_FULL]

    xT = x.T                                                         # view [D_IN, T]
    in_maps = []
    for c in range(N_CORES):
        sl = slice(c * T_CORE, (c + 1) * T_CORE)
        in_maps.append({
            "xT": np.ascontiguousarray(xT[:, sl]),
            "wT": wT,
            "aT": aT,
            "bT": bT,
            "maskM": np.ascontiguousarray(mask[:, sl]),
            "bias_rep": bias_rep,
        })
    return in_maps


def kernel(x, base_weight, base_bias, lora_a, lora_b, token_lora_indices):
    nc = _build()
    in_maps = _prep_inputs(x, base_weight, base_bias, lora_a, lora_b,
                           token_lora_indices)
    res = run_bass_kernel_spmd(nc, in_maps, list(range(N_CORES)))
    return np.concatenate([res.results[c]["out"] for c in range(N_CORES)], axis=0)



# revision 10
# speedup vs baseline: 87.9004x; 87.9004x over previous
"""Trainium2 Bass kernel for BaseLinearLayerWithLoRA (moe_routing).

out = x @ W^T + b  +  per-token LoRA:  out[t] += (x[t] @ A[l]^T) @ B[l]^T,  l = idx[t]

Sharding: data-parallel over tokens across 8 NeuronCores (4096 tokens each);
W, bias and the stacked LoRA A/B tables are replicated.

Per-core kernel design (single pass over tokens):
  - Inputs x and W are host-cast to bf16 (full-rate PE, half the HBM
    traffic of fp32; rms error ~2e-3 vs the 2e-2 gate).  lhsT = x^T chunks
    (host-transposed x), rhs = W^T (host-transposed W) fully resident in
    SBUF; x is streamed once in 256-token super-blocks, with a 4-wide
    o-sweep per stationary chunk into 4 PSUM banks.
  - LoRA shrink: S^T[r,t] = sum_d A_all^T[d,r] x^T[d,t] for all 8 adapters
    at once (A_all = stacked [128, 2048]), one GEMM per super-block.
    A host-precomputed one-hot mask (mask[r,t] = r//16==idx[t]) zeroes the
    rows of foreign adapters per token column (DVE multiply, cast to bf16).
    The expand is then a single bf16 matmul per output tile contracting all
    128 adapter-rank rows, accumulated into the same PSUM tile as the base
    GEMM.
  - Bias is added during the PSUM->SBUF drain (host-replicated to 128 rows,
    cast to bf16 on load).
"""

import sys

for _p in ("/opt/trn_rl_repo", "/root/.axon_site/_ro/trn_rl_repo"):
    if _p not in sys.path:
        sys.path.insert(0, _p)

import numpy as np
import ml_dtypes

import concourse.bass as bass  # noqa: F401  (registers engines)
import concourse.mybir as mybir
import concourse.tile as tile
from concourse import bacc
from concourse.bass_utils import run_bass_kernel_spmd

N_CORES = 8
T_FULL, D_IN, D_OUT = 32768, 2048, 2048
MAX_LORAS, RANK = 8, 16
T_CORE = T_FULL // N_CORES          # 4096 tokens per core
SB_T = 256                          # super-block tokens
N_SB = T_CORE // SB_T               # 16 super-blocks
N_BLK = SB_T // 128                 # 2 token blocks per super-block
KC = D_IN // 128                    # 16 contraction chunks
N_OT = D_OUT // 512                 # 4 o-tiles (full width resident)

_CACHED = {}


def _dedup_ldweights(nc):
    """Drop InstLdweights whose stationary matches the PE array's current
    contents (loaded by the previous kept Ldweights with only non-transpose
    Matmults in between).  tile_legalize emits one Ldweights per matmul even
    when consecutive matmuls share the identical stationary (the 4-wide
    o-sweep); on HW each redundant load streams 128 rows through the PE.
    Ldweights carry no sync_info (all semaphores live on the matmuls), so
    removal does not alter the synchronization graph."""
    import concourse.mybir as _mb

    n_dropped = 0
    for blk in nc.m.functions[0].blocks:
        insts = blk.instructions
        keep = []
        last_key = None
        for x in insts:
            nm = type(x).__name__
            if nm == "InstLdweights":
                key = (str(x.ins[0]), str(x.perf_mode), str(x.is_transpose),
                       str(x.tile_position), str(x.tile_size))
                if key == last_key and x.sync_info is None:
                    n_dropped += 1
                    continue
                last_key = key
            elif nm == "InstMatmult":
                if x.is_transpose:
                    last_key = None
            elif getattr(x, "engine", None) == _mb.EngineType.PE:
                last_key = None  # conservative: unknown PE inst clobbers array
            keep.append(x)
        if len(keep) != len(insts):
            insts[:] = keep
    return n_dropped


def _build(reps=1, store=True, xstream=True):
    key = ("nc", reps, store, xstream)
    if key in _CACHED:
        return _CACHED[key]

    f32 = mybir.dt.float32
    bf16 = mybir.dt.bfloat16

    nc = bacc.Bacc("TRN2", target_bir_lowering=False, debug=False)

    xT = nc.dram_tensor("xT", [D_IN, T_CORE], bf16, kind="ExternalInput")
    wT = nc.dram_tensor("wT", [D_IN, D_OUT], bf16, kind="ExternalInput")
    aT = nc.dram_tensor("aT", [D_IN, 128], bf16, kind="ExternalInput")
    bT = nc.dram_tensor("bT", [128, D_OUT], bf16, kind="ExternalInput")
    maskM = nc.dram_tensor("maskM", [128, T_CORE], bf16, kind="ExternalInput")
    bias_rep = nc.dram_tensor("bias_rep", [128, D_OUT], f32, kind="ExternalInput")
    out = nc.dram_tensor("out", [T_CORE, D_OUT], f32, kind="ExternalOutput")

    xT_v = xT.rearrange("(c p) t -> p c t", p=128)      # [128, 16, T_CORE]
    wT_v = wT.rearrange("(c p) o -> p c o", p=128)      # [128, 16, 2048]
    aT_v = aT.rearrange("(c p) r -> p c r", p=128)      # [128, 16, 128]

    with tile.TileContext(nc) as tc:
        with (
            tc.tile_pool(name="const", bufs=1) as const,
            tc.tile_pool(name="wpool", bufs=min(2, reps)) as wpool,
            tc.tile_pool(name="xpool", bufs=3) as xpool,
            tc.tile_pool(name="mpool", bufs=3) as mpool,
            tc.tile_pool(name="opool", bufs=3) as opool,
            tc.tile_pool(name="pso", bufs=8, space="PSUM") as pso,
        ):
            at = const.tile([128, KC, 128], bf16)
            bt = const.tile([128, D_OUT], bf16)
            bias_t = const.tile([128, D_OUT], bf16)
            for _rep in range(reps):
                wt = wpool.tile([128, KC, D_OUT], bf16, tag="wt", name="wt")
                for c in range(KC):
                    # W loads ride the second HWDGE ring (ACT) so they don't
                    # head-of-line-block the x/mask stream on the SP ring.
                    nc.scalar.dma_start(wt[:, c, :], wT_v[:, c, :])
                for s in range(N_SB):
                    t0 = s * SB_T
                    if xstream or (s == 0 and _rep == 0):
                        xt = xpool.tile([128, KC, SB_T], bf16, tag="xt", name="xt")
                        # per-block pieces so block-0 matmuls can start early
                        for b4 in range(N_BLK):
                            nc.sync.dma_start(
                                xt[:, :, b4 * 128:(b4 + 1) * 128],
                                xT_v[:, :, t0 + b4 * 128:t0 + (b4 + 1) * 128])
                    mk = mpool.tile([128, SB_T], bf16, tag="mk", name="mk")
                    if s == 0 and _rep == 0:
                        nc.scalar.dma_start(at[:], aT_v[:])
                    nc.scalar.dma_start(mk[:], maskM[:, t0:t0 + SB_T])
                    if s == 0 and _rep == 0:
                        nc.scalar.dma_start(bt[:], bT[:])
                        nc.gpsimd.dma_start(bias_t[:], bias_rep[:])  # SWDGE cast
                    s_m = mpool.tile([128, SB_T], bf16, tag="s_m", name="s_m")
                    for b in range(N_BLK):
                        tb = b * 128
                        psums = [
                            pso.tile([128, 512], f32, tag="ps_o", name=f"ps_o{o}")
                            for o in range(N_OT)
                        ]
                        for c in range(KC):
                            for o in range(N_OT):
                                nc.tensor.matmul(
                                    psums[o][:],
                                    xt[:, c, tb:tb + 128],
                                    wt[:, c, o * 512:(o + 1) * 512],
                                    start=(c == 0), stop=False)
                        if b == 0:
                            # LoRA shrink for the whole super-block (all
                            # adapters), emitted after block-0 base matmuls so
                            # the PE has work while at/mask are in flight.
                            ps_s = pso.tile([128, 512], f32, tag="ps_o", name="ps_s")
                            for c in range(KC):
                                nc.tensor.matmul(ps_s[:, :SB_T], at[:, c, :],
                                                 xt[:, c, :],
                                                 start=(c == 0), stop=(c == KC - 1))
                            nc.vector.tensor_tensor(s_m[:], ps_s[:, :SB_T], mk[:],
                                                    mybir.AluOpType.mult)
                        for o in range(N_OT):
                            nc.tensor.matmul(
                                psums[o][:],
                                s_m[:, tb:tb + 128],
                                bt[:, o * 512:(o + 1) * 512],
                                start=False, stop=True)
                        half = D_OUT // 2
                        ot = opool.tile([128, half], f32, tag="ot", name="ot")
                        for o in range(2):
                            nc.vector.tensor_tensor(
                                ot[:, o * 512:(o + 1) * 512], psums[o][:],
                                bias_t[:, o * 512:(o + 1) * 512],
                                mybir.AluOpType.add)
                        if store:
                            nc.gpsimd.dma_start(out[t0 + tb:t0 + tb + 128, :half], ot[:])
                        ot2 = opool.tile([128, half], f32, tag="ot", name="ot2")
                        for o in range(2, 4):
                            nc.vector.tensor_tensor(
                                ot2[:, (o - 2) * 512:(o - 1) * 512], psums[o][:],
                                bias_t[:, o * 512:(o + 1) * 512],
                                mybir.AluOpType.add)
                        if store:
                            nc.gpsimd.dma_start(out[t0 + tb:t0 + tb + 128, half:], ot2[:])

    _dedup_ldweights(nc)
    nc.compile()
    _CACHED[key] = nc
    return nc


TF_CORE = 4352                      # fold-path tokens per core (17 super-blocks)
NF_SB = TF_CORE // SB_T


def _build_fold(reps=1):
    """Adapter-sharded pure-GEMM kernel: each core computes
    out = x_l @ W_l^T + b for one adapter's (padded) token set, with
    W_l = W + B_l A_l folded on the host.  No on-device LoRA at all."""
    key = ("fold", reps)
    if key in _CACHED:
        return _CACHED[key]

    f32 = mybir.dt.float32
    bf16 = mybir.dt.bfloat16

    nc = bacc.Bacc("TRN2", target_bir_lowering=False, debug=False)

    xT = nc.dram_tensor("xT", [D_IN, TF_CORE], bf16, kind="ExternalInput")
    wT = nc.dram_tensor("wT", [D_IN, D_OUT], bf16, kind="ExternalInput")
    bias_rep = nc.dram_tensor("bias_rep", [128, D_OUT], f32, kind="ExternalInput")
    out = nc.dram_tensor("out", [TF_CORE, D_OUT], f32, kind="ExternalOutput")

    xT_v = xT.rearrange("(c p) t -> p c t", p=128)      # [128, 16, TF_CORE]
    wT_v = wT.rearrange("(c p) o -> p c o", p=128)      # [128, 16, 2048]

    with tile.TileContext(nc) as tc:
        with (
            tc.tile_pool(name="const", bufs=1) as const,
            tc.tile_pool(name="wpool", bufs=min(2, reps)) as wpool,
            tc.tile_pool(name="xpool", bufs=3) as xpool,
            tc.tile_pool(name="opool", bufs=3) as opool,
            tc.tile_pool(name="pso", bufs=8, space="PSUM") as pso,
        ):
            bias_t = const.tile([128, D_OUT], bf16)
            for _rep in range(reps):
                wt = wpool.tile([128, KC, D_OUT], bf16, tag="wt", name="wt")
                for c in range(KC):
                    nc.scalar.dma_start(wt[:, c, :], wT_v[:, c, :])
                if _rep == 0:
                    nc.gpsimd.dma_start(bias_t[:], bias_rep[:])  # SWDGE cast
                for s in range(NF_SB):
                    t0 = s * SB_T
                    xt = xpool.tile([128, KC, SB_T], bf16, tag="xt", name="xt")
                    for b4 in range(N_BLK):
                        nc.sync.dma_start(
                            xt[:, :, b4 * 128:(b4 + 1) * 128],
                            xT_v[:, :, t0 + b4 * 128:t0 + (b4 + 1) * 128])
                    for b in range(N_BLK):
                        tb = b * 128
                        psums = [
                            pso.tile([128, 512], f32, tag="ps_o", name=f"ps_o{o}")
                            for o in range(N_OT)
                        ]
                        for c in range(KC):
                            for o in range(N_OT):
                                nc.tensor.matmul(
                                    psums[o][:],
                                    xt[:, c, tb:tb + 128],
                                    wt[:, c, o * 512:(o + 1) * 512],
                                    start=(c == 0), stop=(c == KC - 1))
                        half = D_OUT // 2
                        ot = opool.tile([128, half], f32, tag="ot", name="ot")
                        for o in range(2):
                            nc.vector.tensor_tensor(
                                ot[:, o * 512:(o + 1) * 512], psums[o][:],
                                bias_t[:, o * 512:(o + 1) * 512],
                                mybir.AluOpType.add)
                        nc.gpsimd.dma_start(out[t0 + tb:t0 + tb + 128, :half], ot[:])
                        ot2 = opool.tile([128, half], f32, tag="ot", name="ot2")
                        for o in range(2, 4):
                            nc.vector.tensor_tensor(
                                ot2[:, (o - 2) * 512:(o - 1) * 512], psums[o][:],
                                bias_t[:, o * 512:(o + 1) * 512],
                                mybir.AluOpType.add)
                        nc.gpsimd.dma_start(out[t0 + tb:t0 + tb + 128, half:], ot2[:])

    _dedup_ldweights(nc)
    nc.compile()
    _CACHED[key] = nc
    return nc


def _prep_fold(x, base_weight, base_bias, lora_a, lora_b, token_lora_indices):
    """Host prep for the adapter-sharded fold path.  Returns (in_maps,
    token_lists) or None if some adapter has more than TF_CORE tokens."""
    bf16 = ml_dtypes.bfloat16
    x = np.asarray(x, dtype=np.float32)
    w = np.asarray(base_weight, dtype=np.float32)
    bias = np.asarray(base_bias, dtype=np.float32)
    la = np.asarray(lora_a, dtype=np.float32)
    lb = np.asarray(lora_b, dtype=np.float32)
    idx = np.asarray(token_lora_indices, dtype=np.int32)

    toks = [np.nonzero(idx == l)[0] for l in range(MAX_LORAS)]
    if max(len(t) for t in toks) > TF_CORE:
        return None

    bias_rep = np.ascontiguousarray(
        np.broadcast_to(bias[None, :], (128, D_OUT)))
    xT_all = np.ascontiguousarray(x.T).astype(bf16)      # [D_IN, T_FULL]

    in_maps = []
    for l in range(MAX_LORAS):
        wl = w + lb[l, 0] @ la[l, 0]                     # [D_OUT, D_IN] f32
        wTl = np.ascontiguousarray(wl.T).astype(bf16)    # [D_IN, D_OUT]
        xTl = np.zeros((D_IN, TF_CORE), dtype=bf16)
        tl = toks[l]
        xTl[:, :len(tl)] = xT_all[:, tl]
        in_maps.append({"xT": xTl, "wT": wTl, "bias_rep": bias_rep})
    return in_maps, toks


def kernel(x, base_weight, base_bias, lora_a, lora_b, token_lora_indices):
    # masked-LoRA data-parallel path (measured faster on HW than the
    # adapter-sharded folded-W variant, see _build_fold)
    nc = _build()
    in_maps = _prep_inputs(x, base_weight, base_bias, lora_a, lora_b,
                           token_lora_indices)
    res = run_bass_kernel_spmd(nc, in_maps, list(range(N_CORES)))
    return np.concatenate([res.results[c]["out"] for c in range(N_CORES)], axis=0)


def _prep_inputs(x, base_weight, base_bias, lora_a, lora_b, token_lora_indices):
    bf16 = ml_dtypes.bfloat16
    x = np.asarray(x, dtype=np.float32)
    w = np.asarray(base_weight, dtype=np.float32)
    bias = np.asarray(base_bias, dtype=np.float32)
    la = np.asarray(lora_a, dtype=np.float32)
    lb = np.asarray(lora_b, dtype=np.float32)
    idx = np.asarray(token_lora_indices, dtype=np.int32)

    wT = np.ascontiguousarray(w.T).astype(bf16)                      # [D_IN, D_OUT]
    aT = np.ascontiguousarray(la.reshape(128, D_IN).T).astype(bf16)  # [D_IN, 128]
    bT = np.ascontiguousarray(
        lb[:, 0].transpose(0, 2, 1).reshape(128, D_OUT)).astype(bf16)
    bias_rep = np.ascontiguousarray(
        np.broadcast_to(bias[None, :], (128, D_OUT)))                # [128, D_OUT]
    mask = (np.arange(128, dtype=np.int32)[:, None] // RANK
            == idx[None, :]).astype(bf16)                            # [128, T_FULL]

    xT = x.T.astype(bf16)                                            # [D_IN, T]
    in_maps = []
    for c in range(N_CORES):
        sl = slice(c * T_CORE, (c + 1) * T_CORE)
        in_maps.append({
            "xT": np.ascontiguousarray(xT[:, sl]),
            "wT": wT,
            "aT": aT,
            "bT": bT,
            "maskM": np.ascontiguousarray(mask[:, sl]),
            "bias_rep": bias_rep,
        })
    return in_maps
